# revision 45
# baseline (speedup 1.0000x reference)
import numpy as np
import concourse.bass as bass
import concourse.mybir as mybir
from concourse.bass_utils import run_bass_kernel_spmd
from concourse.tile import TileContext
from concourse.mybir import AluOpType as Alu, ActivationFunctionType as Act
from concourse.mybir import MatmulPerfMode

B, T, D, H, hd, SC, ST = 2, 2048, 1024, 16, 64, 64, 16
BT = B * T          # 4096
NC = 8
TOK = BT // NC      # 512 tokens per core (256 from each batch)
HTOK = 256
EPS = 1.1920929e-07
F32 = mybir.dt.float32
BF16 = mybir.dt.bfloat16
FP8 = mybir.dt.float8e4
WS = 32.0  # fp8 weight/data scale; descales fold into exp-scale & residual add


def _split_multi_waits(nc, max_waits=1):
    # this walrus build accepts only one sync wait per ISA instruction
    n = 0
    for f in nc.m.functions:
        for bb in f.blocks:
            out = []
            for inst in bb.instructions:
                si = inst.sync_info
                if si is not None and si.on_wait and len(si.on_wait) > max_waits:
                    for w in si.on_wait[:-max_waits]:
                        out.append(mybir.InstNoOp(
                            name=f"{inst.name}_ws{n}", ins=[], outs=[],
                            engine=inst.engine,
                            sync_info=mybir.SyncInfo(on_wait=[w], on_update=[]),
                            bass_nofuse=True))
                        n += 1
                    inst.sync_info = mybir.SyncInfo(
                        on_wait=si.on_wait[-max_waits:], on_update=si.on_update)
                out.append(inst)
            bb.instructions = out
    return n


def _build():
    nc = bass.Bass()

    xTb = nc.dram_tensor("xTb", [D, BT], BF16, kind="ExternalInput")
    x_myT = nc.dram_tensor("x_myT", [D, TOK], F32, kind="ExternalInput")
    qkwT8 = nc.dram_tensor("qkwT8", [D, 256], FP8, kind="ExternalInput")
    vwT = nc.dram_tensor("vwT", [D, 128], BF16, kind="ExternalInput")
    o_wT = nc.dram_tensor("o_wT", [D, D], BF16, kind="ExternalInput")
    out_wT = nc.dram_tensor("out_wT", [SC, D], BF16, kind="ExternalInput")
    in_wT = nc.dram_tensor("in_wT", [D, SC], BF16, kind="ExternalInput")
    gate_wT = nc.dram_tensor("gate_wT", [D, SC], BF16, kind="ExternalInput")
    dt_wT = nc.dram_tensor("dt_wT", [SC, SC], BF16, kind="ExternalInput")
    BpT = nc.dram_tensor("BpT", [SC, ST], BF16, kind="ExternalInput")
    CpT = nc.dram_tensor("CpT", [SC, ST], BF16, kind="ExternalInput")
    w1c = nc.dram_tensor("w1c", [128, 8], F32, kind="ExternalInput")
    w2c = nc.dram_tensor("w2c", [128, 8], F32, kind="ExternalInput")
    dtb = nc.dram_tensor("dtb", [SC, 1], F32, kind="ExternalInput")
    alog = nc.dram_tensor("alog", [128, 8], F32, kind="ExternalInput")
    ident = nc.dram_tensor("ident", [128, 128], BF16, kind="ExternalInput")
    tri = nc.dram_tensor("tri", [128, 128], BF16, kind="ExternalInput")
    onesd = nc.dram_tensor("onesd", [128, 512], BF16, kind="ExternalInput")
    esc = nc.dram_tensor("esc", [SC, 1024], BF16, kind="ExternalInput")
    est = nc.dram_tensor("est", [ST, 128], BF16, kind="ExternalInput")
    r8 = nc.dram_tensor("r8", [128, 512], BF16, kind="ExternalInput")
    dsel = nc.dram_tensor("dsel", [16, 1024], BF16, kind="ExternalInput")
    csel = nc.dram_tensor("csel", [128, 8], F32, kind="ExternalInput")
    omc = nc.dram_tensor("omc", [128, 128], F32, kind="ExternalInput")
    epsb = nc.dram_tensor("epsb", [128, 1], F32, kind="ExternalInput")

    yout = nc.dram_tensor("yout", [D, TOK], F32, kind="ExternalOutput")

    with nc.allow_low_precision(reason="bf16 within tolerance"), \
         TileContext(nc) as tc:
        with tc.tile_pool(name="const", bufs=1) as cpool, \
             tc.tile_pool(name="wts", bufs=1) as wpool, \
             tc.tile_pool(name="big", bufs=1) as bigp, \
             tc.tile_pool(name="work", bufs=2) as work, \
             tc.tile_pool(name="scan", bufs=1) as spool, \
             tc.tile_pool(name="psA", bufs=2, space="PSUM") as psA, \
             tc.tile_pool(name="psB", bufs=3, space="PSUM") as psB, \
             tc.tile_pool(name="psC", bufs=2, space="PSUM") as psC, \
             tc.tile_pool(name="psV", bufs=1, space="PSUM") as psV, \
             tc.tile_pool(name="dram", bufs=1, space="DRAM") as dram:

            def csbuf(shape, src, name, d=BF16):
                t = cpool.tile(shape, d, name=name, tag=name)
                nc.sync.dma_start(t[:, :], src)
                return t

            # ---- first-needed: x block (b0,blk0) + rms/qkv consts ----
            xt0 = [work.tile([128, 512], BF16, name=f"xt{i}", tag="xt", bufs=16)
                   for i in range(8)]
            for k in range(8):
                nc.sync.dma_start(xt0[k][:, :], xTb[k * 128:(k + 1) * 128, 0:512])
            onesS = csbuf([128, 512], onesd[:, :], "onesS")
            epsS = csbuf([128, 1], epsb[:, :], "epsS", F32)
            identS = csbuf([128, 128], ident[:, :], "identS")
            qkvW = wpool.tile([128, 8, 256], FP8, name="qkvW", tag="qkvW")
            vW = wpool.tile([128, 8 * 128], BF16, name="vW", tag="vW")
            for k in range(8):
                nc.sync.dma_start(qkvW[:, k:k + 1, :],
                                  qkwT8[k * 128:(k + 1) * 128, :])
                nc.sync.dma_start(vW[:, k * 128:(k + 1) * 128],
                                  vwT[k * 128:(k + 1) * 128, :])
            triS = csbuf([128, 128], tri[:, :], "triS")

            cinA = dram.tile([NC, 130, HTOK], BF16, name="cinA", tag="cinA")
            coutA = dram.tile([NC, 130, HTOK], BF16, name="coutA", tag="coutA")
            cinB = dram.tile([NC, 130, HTOK], BF16, name="cinB", tag="cinB")
            coutB = dram.tile([NC, 130, HTOK], BF16, name="coutB", tag="coutB")

            Qf = {}
            Kf = {}
            Vp = {}

            def qkv_block(b, blk, first=False):
                if blk == 0:
                    Qf[b] = bigp.tile([128, T], BF16, name=f"Qf{b}", tag=f"Qf{b}", bufs=1)
                    Kf[b] = bigp.tile([128, T], BF16, name=f"Kf{b}", tag=f"Kf{b}", bufs=1)
                    Vp[b] = [[bigp.tile([128, 65], BF16, name=f"Vp{b}_{hh}_{kt}",
                                        tag=f"Vp{b}_{hh}_{kt}", bufs=1)
                              for kt in range(16)] for hh in range(2)]
                    for hh in range(2):
                        for kt in range(16):
                            nc.vector.tensor_copy(Vp[b][hh][kt][:, 64:65],
                                                  onesS[:, 0:1])
                c0 = b * T + blk * 512
                if first:
                    xt = xt0
                else:
                    xt = [work.tile([128, 512], BF16, name=f"xt{i}", tag="xt",
                                    bufs=16) for i in range(8)]
                    for k in range(8):
                        nc.sync.dma_start(xt[k][:, :],
                                          xTb[k * 128:(k + 1) * 128, c0:c0 + 512])
                # sum of squares: vector squares + pairwise add tree
                sq = [work.tile([128, 512], BF16, name=f"sq{i}", tag=f"sq{i}", bufs=2)
                      for i in range(4)]
                for i in range(4):
                    nc.vector.tensor_mul(sq[i][:, :], xt[2 * i][:, :],
                                         xt[2 * i][:, :])
                    sqb = work.tile([128, 512], BF16, tag="sqb", bufs=2)
                    nc.vector.tensor_mul(sqb[:, :], xt[2 * i + 1][:, :],
                                         xt[2 * i + 1][:, :])
                    nc.vector.tensor_add(sq[i][:, :], sq[i][:, :], sqb[:, :])
                nc.vector.tensor_add(sq[0][:, :], sq[0][:, :], sq[1][:, :])
                nc.vector.tensor_add(sq[2][:, :], sq[2][:, :], sq[3][:, :])
                nc.vector.tensor_add(sq[0][:, :], sq[0][:, :], sq[2][:, :])
                ssp = psA.tile([1, 512], F32, tag="psA")
                nc.tensor.matmul(ssp[:, :], onesS[:, 0:1], sq[0][:, :],
                                 start=True, stop=True)
                # rsqrt(ms) = exp(-0.5 * ln(ms + eps)) — stays in exp/ln table
                lnr = work.tile([1, 512], F32, tag="lnr")
                nc.scalar.activation(lnr[:, :], ssp[:, :], Act.Ln,
                                     scale=1.0 / D, bias=epsS[0:1, :])
                rs = work.tile([1, 512], BF16, tag="rs")
                nc.scalar.activation(rs[:, :], lnr[:, :], Act.Exp, scale=-0.5)
                rsbp = psA.tile([128, 512], F32, tag="psA")
                nc.tensor.matmul(rsbp[:, :], onesS[0:1, 0:128], rs[:, :],
                                 start=True, stop=True)
                rsb = work.tile([128, 512], BF16, tag="rsbc", bufs=2)
                nc.scalar.copy(rsb[:, :], rsbp[:, :])
                # per-token rsqrt transposed to partitions (folds into Vp)
                rsP = psV.tile([128, 8], BF16, tag="psV")
                rsT = work.tile([128, 4], F32, tag="rsT", bufs=2)
                for sub in range(4):
                    nc.tensor.transpose(rsP[:, 2 * sub:2 * sub + 1],
                                        rs[0:1, 128 * sub:128 * (sub + 1)],
                                        identS[0:1, 0:1])
                    nc.vector.tensor_copy(rsT[:, sub:sub + 1],
                                          rsP[:, 2 * sub:2 * sub + 1])
                xt8 = [work.tile([128, 2, 512], FP8, name=f"xt8{i}",
                                 tag=f"xt8{i}", bufs=2) for i in range(4)]
                for k in range(8):
                    nc.vector.tensor_mul(xt8[k // 2][:, k % 2, :],
                                         xt[k][:, :], rsb[:, :])
                # Q,K: fp8 double-pumped on normalized x
                for m in range(2):
                    om = psB.tile([128, 512], F32, tag="psB")
                    for kk in range(4):
                        nc.tensor.matmul(
                            om[:, :],
                            qkvW[:, 2 * kk:2 * kk + 2, m * 128:(m + 1) * 128],
                            xt8[kk][:, :, :], start=(kk == 0), stop=(kk == 3),
                            perf_mode=MatmulPerfMode.DoubleRow)
                    dst = Qf[b] if m == 0 else Kf[b]
                    nc.scalar.copy(dst[:, blk * 512:(blk + 1) * 512], om[:, :])
                # V: bf16 from RAW x; the rsqrt factor is folded into Vp
                om = psB.tile([128, 512], F32, tag="psB")
                for k in range(8):
                    nc.tensor.matmul(om[:, :], vW[:, k * 128:(k + 1) * 128],
                                     xt[k][:, :], start=(k == 0), stop=(k == 7))
                vfb = work.tile([128, 512], BF16, tag="vfb")
                nc.scalar.copy(vfb[:, :], om[:, :])
                for sub in range(4):
                    kt = blk * 4 + sub
                    for hh in range(2):
                        vtp = psV.tile([128, 64], BF16, tag="psV")
                        nc.tensor.transpose(
                            vtp[:, :],
                            vfb[64 * hh:64 * hh + 64,
                                sub * 128:(sub + 1) * 128],
                            identS[64 * hh:64 * hh + 64,
                                   64 * hh:64 * hh + 64])
                        nc.vector.tensor_scalar_mul(Vp[b][hh][kt][:, 0:64],
                                                    vtp[:, :],
                                                    rsT[:, sub:sub + 1])

            def attn_group(b, hh, qb):
                cinX = cinA if b == 0 else cinB
                r0 = 64 * hh
                q0 = qb * 512
                ops = psC.tile([65, 512], F32, tag="psC")
                nkt = 4 * qb + 4

                def score_mm(kt):
                    d = kt - 4 * qb
                    lo = 128 * d if d > 0 else 0
                    sp = psB.tile([128, 512], F32, tag="psB")
                    nc.tensor.matmul(
                        sp[:, lo:512],
                        Kf[b][r0:r0 + 64, kt * 128:(kt + 1) * 128],
                        Qf[b][r0:r0 + 64, q0 + lo:q0 + 512],
                        start=True, stop=True)
                    return sp

                sps = score_mm(0)
                for kt in range(nkt):
                    sp = sps
                    if kt + 1 < nkt:
                        sps = score_mm(kt + 1)
                    e = work.tile([128, 512], BF16, tag="expst", bufs=3)
                    d = kt - 4 * qb
                    if d < 0:
                        nc.scalar.activation(e[:, :], sp[:, :], Act.Exp,
                                             scale=0.125 / (WS * WS))
                        nc.tensor.matmul(ops[:, :], Vp[b][hh][kt][:, :],
                                         e[:, :], start=(kt == 0),
                                         stop=(kt == nkt - 1),
                                         skip_group_check=True)
                    else:
                        # keys here only attend to queries >= 128*d
                        nc.scalar.activation(e[:, 128 * d:512],
                                             sp[:, 128 * d:512],
                                             Act.Exp, scale=0.125 / (WS * WS))
                        nc.vector.tensor_mul(
                            e[:, 128 * d:128 * (d + 1)],
                            e[:, 128 * d:128 * (d + 1)], triS[:, :])
                        nc.tensor.matmul(ops[:, 128 * d:512],
                                         Vp[b][hh][kt][:, :],
                                         e[:, 128 * d:512],
                                         start=(kt == 0),
                                         stop=(kt == nkt - 1),
                                         skip_group_check=True)
                # ship unnormalized PV sums + denominators; queries q0..q0+512
                # of batch b belong to cores 2qb (first 256) and 2qb+1
                rbs = work.tile([65, 512], BF16, tag="rbs")
                nc.vector.tensor_copy(rbs[:, :], ops[:, :])
                for s in range(2):
                    dst = 2 * qb + s
                    nc.gpsimd.dma_start(
                        cinX[dst, r0:r0 + 64, :],
                        rbs[0:64, 256 * s:256 * s + 256])
                    nc.gpsimd.dma_start(
                        cinX[dst, 128 + hh:129 + hh, :],
                        rbs[64:65, 256 * s:256 * s + 256])

            # ---- emission: qkv b0, then attention b0 with qkv b1 woven in ----
            for blk in range(4):
                qkv_block(0, blk, first=(blk == 0))
            seq = [("A", 0, 0, 0), ("A", 0, 0, 1), ("Q", 1, 0),
                   ("A", 0, 0, 2), ("Q", 1, 1), ("A", 0, 0, 3), ("Q", 1, 2),
                   ("A", 0, 1, 0), ("Q", 1, 3), ("A", 0, 1, 1),
                   ("A", 0, 1, 2), ("A", 0, 1, 3)]
            for item in seq:
                if item[0] == "A":
                    attn_group(item[1], item[2], item[3])
                else:
                    qkv_block(item[1], item[2])

            nc.gpsimd.collective_compute(
                "AllToAll", Alu.bypass, [list(range(NC))],
                ins=[cinA.opt()], outs=[coutA.opt()])

            # deferred weights/consts (all consumed post-collective); they
            # load during the attention phases without delaying x blocks
            owS = wpool.tile([128, 8 * 1024], BF16, name="owS", tag="owS")
            for k in range(8):
                nc.sync.dma_start(owS[:, k * 1024:(k + 1) * 1024],
                                  o_wT[k * 128:(k + 1) * 128, :])
            escS = csbuf([SC, 1024], esc[:, :], "escS")
            estS = csbuf([ST, 128], est[:, :], "estS")
            r8S = csbuf([128, 512], r8[:, :], "r8S")
            dselS = csbuf([16, 1024], dsel[:, :], "dselS")
            cselS = csbuf([128, 8], csel[:, :], "cselS", F32)
            omcS = csbuf([128, 128], omc[:, :], "omcS", F32)
            dtbS = csbuf([SC, 1], dtb[:, :], "dtbS", F32)
            alogS = csbuf([128, 8], alog[:, :], "alogS", F32)
            inW = wpool.tile([128, 8 * SC], BF16, name="inW", tag="inW")
            gateW = wpool.tile([128, 8 * SC], BF16, name="gateW", tag="gateW")
            for k in range(8):
                nc.sync.dma_start(inW[:, k * SC:(k + 1) * SC],
                                  in_wT[k * 128:(k + 1) * 128, :])
                nc.sync.dma_start(gateW[:, k * SC:(k + 1) * SC],
                                  gate_wT[k * 128:(k + 1) * 128, :])
            outW = wpool.tile([SC, D], BF16, name="outW", tag="outW")
            nc.sync.dma_start(outW[:, :], out_wT[:, :])
            dtW = wpool.tile([SC, SC], BF16, name="dtW", tag="dtW")
            nc.sync.dma_start(dtW[:, :], dt_wT[:, :])
            BpS = wpool.tile([SC, ST], BF16, name="BpS", tag="BpS")
            nc.sync.dma_start(BpS[:, :], BpT[:, :])
            CpS = wpool.tile([SC, ST], BF16, name="CpS", tag="CpS")
            nc.sync.dma_start(CpS[:, :], CpT[:, :])
            negA = cpool.tile([128, 8], F32, name="negA", tag="negA")
            nc.scalar.activation(negA[:, :], alogS[:, :], Act.Exp)
            nc.vector.tensor_scalar_mul(negA[:, :], negA[:, :], -1.0)

            for hh in range(2):
                for qb in range(4):
                    attn_group(1, hh, qb)

            nc.gpsimd.collective_compute(
                "AllToAll", Alu.bypass, [list(range(NC))],
                ins=[cinB.opt()], outs=[coutB.opt()])

            # ======== per-half tail: o_proj → rms2 → proj → scan p1 ========
            cin2 = dram.tile([128, 32], BF16, name="cin2T", tag="cin2T")
            cout2 = dram.tile([NC * 128, 32], BF16, name="cout2T", tag="cout2T")
            xmy = [bigp.tile([128, 512], F32, name=f"xmy{m}", tag=f"xmy{m}",
                             bufs=1) for m in range(8)]
            for m in range(8):
                nc.sync.dma_start(xmy[m][:, :], x_myT[m * 128:(m + 1) * 128, :])
            x1h = xmy  # accumulate in place (fp32)
            h2T = [bigp.tile([128, 512], BF16, name=f"h2Tn{k}", tag=f"h2Tn{k}",
                             bufs=1) for k in range(8)]
            z_s = spool.tile([SC, 512], BF16, tag="z_s")
            gate_p = spool.tile([SC, 512], F32, tag="gate_p")
            dt_s = spool.tile([SC, 512], BF16, tag="dt_s")
            dtz_s = spool.tile([SC, 512], BF16, tag="dtz_s")
            bi_s = spool.tile([ST, 512], BF16, tag="bi_s")
            ci_s = spool.tile([ST, 512], BF16, tag="ci_s")
            bes = spool.tile([128, 512], BF16, tag="bes")
            ces = spool.tile([128, 512], BF16, tag="ces")
            ppT = [spool.tile([128, 512], BF16, name=f"ppT{g}", tag=f"ppT{g}")
                   for g in range(8)]
            sc0T = [spool.tile([128, 512], BF16, name=f"sc0T{g}", tag=f"sc0T{g}")
                    for g in range(8)]

            def tail_half(half, coutX):
                hc = slice(256 * half, 256 * half + 256)
                cog = [bigp.tile([128, HTOK], BF16, name=f"cog{half}_{k}",
                                 tag=f"cog{half}_{k}", bufs=1) for k in range(8)]
                for k in range(8):
                    nc.sync.dma_start(cog[k][:, :], coutX[k, 0:128, :])
                d16 = spool.tile([16, HTOK], BF16, name=f"d16_{half}",
                                 tag=f"d16_{half}")
                for k in range(8):
                    nc.sync.dma_start(d16[2 * k:2 * k + 2, :],
                                      coutX[k, 128:130, :])
                # 1/denom = exp(-ln(denom))
                lnd = work.tile([16, HTOK], F32, tag="lnd")
                nc.scalar.activation(lnd[:, :], d16[:, :], Act.Ln)
                d16r = spool.tile([16, HTOK], BF16, name=f"d16r_{half}",
                                  tag=f"d16r_{half}")
                nc.scalar.activation(d16r[:, :], lnd[:, :], Act.Exp, scale=-1.0)
                for k in range(8):
                    dbk = psA.tile([128, HTOK], F32, tag="psA")
                    nc.tensor.matmul(dbk[:, :], dselS[:, k * 128:(k + 1) * 128],
                                     d16r[:, :], start=True, stop=True)
                    nc.vector.tensor_mul(cog[k][:, :], cog[k][:, :], dbk[:, :])
                for m in range(8):
                    pr = psB.tile([128, HTOK], F32, tag="psB")
                    for k in range(8):
                        nc.tensor.matmul(
                            pr[:, :],
                            owS[:, k * 1024 + m * 128:k * 1024 + (m + 1) * 128],
                            cog[k][:, :], start=(k == 0), stop=(k == 7))
                    nc.vector.tensor_add(x1h[m][:, hc], x1h[m][:, hc], pr[:, :])
                # rmsnorm2 for this half
                sqh = [work.tile([128, HTOK], BF16, name=f"sqh{i}",
                                 tag=f"sq{i}", bufs=2) for i in range(4)]
                for i in range(4):
                    nc.vector.tensor_mul(sqh[i][:, :], x1h[2 * i][:, hc],
                                         x1h[2 * i][:, hc])
                    sqb = work.tile([128, HTOK], BF16, tag="sqb", bufs=2)
                    nc.vector.tensor_mul(sqb[:, :], x1h[2 * i + 1][:, hc],
                                         x1h[2 * i + 1][:, hc])
                    nc.vector.tensor_add(sqh[i][:, :], sqh[i][:, :], sqb[:, :])
                nc.vector.tensor_add(sqh[0][:, :], sqh[0][:, :], sqh[1][:, :])
                nc.vector.tensor_add(sqh[2][:, :], sqh[2][:, :], sqh[3][:, :])
                nc.vector.tensor_add(sqh[0][:, :], sqh[0][:, :], sqh[2][:, :])
                ssp2 = psA.tile([1, HTOK], F32, tag="psA")
                nc.tensor.matmul(ssp2[:, :], onesS[:, 0:1], sqh[0][:, :],
                                 start=True, stop=True)
                lnr2 = work.tile([1, HTOK], F32, tag="lnr")
                nc.scalar.activation(lnr2[:, :], ssp2[:, :], Act.Ln,
                                     scale=1.0 / D, bias=epsS[0:1, :])
                rs2 = work.tile([1, HTOK], BF16, tag="rs")
                nc.scalar.activation(rs2[:, :], lnr2[:, :], Act.Exp, scale=-0.5)
                rsb2p = psA.tile([128, HTOK], F32, tag="psA")
                nc.tensor.matmul(rsb2p[:, :], onesS[0:1, 0:128], rs2[:, :],
                                 start=True, stop=True)
                rsb2 = work.tile([128, HTOK], BF16, tag="rsbc", bufs=2)
                nc.scalar.copy(rsb2[:, :], rsb2p[:, :])
                for k in range(8):
                    nc.vector.tensor_mul(h2T[k][:, hc], x1h[k][:, hc],
                                         rsb2[:, :])
                # projections for this half
                pz = psB.tile([SC, HTOK], F32, tag="psB")
                for k in range(8):
                    nc.tensor.matmul(pz[:, :], inW[:, k * SC:(k + 1) * SC],
                                     h2T[k][:, hc], start=(k == 0), stop=(k == 7))
                nc.vector.tensor_copy(z_s[:, hc], pz[:, :])
                pg = psB.tile([SC, HTOK], F32, tag="psB")
                for k in range(8):
                    nc.tensor.matmul(pg[:, :], gateW[:, k * SC:(k + 1) * SC],
                                     h2T[k][:, hc], start=(k == 0), stop=(k == 7))
                nc.vector.tensor_copy(gate_p[:, hc], pg[:, :])
                pdt = psB.tile([SC, HTOK], F32, tag="psB")
                nc.tensor.matmul(pdt[:, :], dtW[:, :], z_s[:, hc],
                                 start=True, stop=True)
                nc.scalar.activation(dt_s[:, hc], pdt[:, :], Act.Exp,
                                     bias=dtbS[:, :])
                nc.scalar.activation(dt_s[:, hc], dt_s[:, hc], Act.Ln, bias=1.0)
                nc.vector.tensor_mul(dtz_s[:, hc], dt_s[:, hc], z_s[:, hc])
                pbi = psB.tile([ST, HTOK], F32, tag="psB")
                nc.tensor.matmul(pbi[:, :], BpS[:, :], z_s[:, hc],
                                 start=True, stop=True)
                nc.vector.tensor_copy(bi_s[:, hc], pbi[:, :])
                pci = psB.tile([ST, HTOK], F32, tag="psB")
                nc.tensor.matmul(pci[:, :], CpS[:, :], z_s[:, hc],
                                 start=True, stop=True)
                nc.vector.tensor_copy(ci_s[:, hc], pci[:, :])
                pbe = psC.tile([128, HTOK], F32, tag="psC")
                nc.tensor.matmul(pbe[:, :], estS[:, :], bi_s[:, hc],
                                 start=True, stop=True)
                nc.vector.tensor_copy(bes[:, hc], pbe[:, :])
                pce = psC.tile([128, HTOK], F32, tag="psC")
                nc.tensor.matmul(pce[:, :], estS[:, :], ci_s[:, hc],
                                 start=True, stop=True)
                nc.vector.tensor_copy(ces[:, hc], pce[:, :])
                # scan pass 1 for this half
                for g in range(8):
                    pde = psB.tile([128, HTOK], F32, tag="psB")
                    nc.tensor.matmul(pde[:, :], escS[:, g * 128:(g + 1) * 128],
                                     dt_s[:, hc], start=True, stop=True)
                    abar = work.tile([128, HTOK], BF16, tag="abar")
                    nc.vector.scalar_tensor_tensor(abar[:, :], pde[:, :],
                                                   negA[:, g:g + 1],
                                                   onesS[:, 0:HTOK],
                                                   Alu.mult, Alu.add)
                    pdz = psB.tile([128, HTOK], F32, tag="psB")
                    nc.tensor.matmul(pdz[:, :], escS[:, g * 128:(g + 1) * 128],
                                     dtz_s[:, hc], start=True, stop=True)
                    bin_ = work.tile([128, HTOK], BF16, tag="bin_")
                    nc.vector.tensor_mul(bin_[:, :], pdz[:, :], bes[:, hc])
                    nc.vector.tensor_tensor_scan(sc0T[g][:, hc], abar[:, :],
                                                 bin_[:, :], 0.0,
                                                 Alu.mult, Alu.add)
                    nc.vector.tensor_tensor_scan(ppT[g][:, hc], abar[:, :],
                                                 abar[:, :], 1.0,
                                                 Alu.mult, Alu.bypass)
                    last = 256 * half + 255
                    nc.gpsimd.dma_start(
                        cin2[:, 8 * half + g:8 * half + g + 1],
                        ppT[g][:, last:last + 1])
                    nc.gpsimd.dma_start(
                        cin2[:, 16 + 8 * half + g:17 + 8 * half + g],
                        sc0T[g][:, last:last + 1])

            tail_half(0, coutA)
            tail_half(1, coutB)

            nc.gpsimd.collective_compute(
                "AllGather", Alu.bypass, [list(range(NC))],
                ins=[cin2.opt()], outs=[cout2.opt()])

            # seed combine: fold over cores j < my index (same mask both
            # batches); sin[:, 0:8]=batch0 groups, [:, 8:16]=batch1 groups
            pjs = [work.tile([128, 32], BF16, name=f"pj{j}", tag=f"pj{j}",
                             bufs=1) for j in range(NC)]
            for j in range(NC):
                nc.sync.dma_start(pjs[j][:, :], cout2[j * 128:(j + 1) * 128, :])
            # masked per-core terms: p_j (identity 1 when masked), s_j (0)
            ps = []
            for j in range(NC):
                pe_ = work.tile([128, 16], F32, name=f"pe{j}", tag=f"pe{j}",
                                bufs=1)
                nc.vector.scalar_tensor_tensor(pe_[:, :], pjs[j][:, 0:16],
                                               cselS[:, j:j + 1],
                                               omcS[:, 16 * j:16 * (j + 1)],
                                               Alu.mult, Alu.add)
                se_ = work.tile([128, 16], F32, name=f"se{j}", tag=f"se{j}",
                                bufs=1)
                nc.vector.tensor_scalar_mul(se_[:, :], pjs[j][:, 16:32],
                                            cselS[:, j:j + 1])
                ps.append((pe_, se_))
            # tree combine: (p1,s1) then (p2,s2) -> (p1*p2, s1*p2 + s2)
            lvl = ps
            while len(lvl) > 1:
                nxt = []
                for i in range(0, len(lvl), 2):
                    (pa, sa), (pb, sb) = lvl[i], lvl[i + 1]
                    nc.vector.scalar_tensor_tensor(sa[:, :], sa[:, :], 1.0,
                                                   sb[:, :], Alu.mult_rhs := Alu.mult, Alu.add)                         if False else None
                    # s = sa*pb + sb ; p = pa*pb  (in place into a-side)
                    nc.vector.tensor_mul(sa[:, :], sa[:, :], pb[:, :])
                    nc.vector.tensor_add(sa[:, :], sa[:, :], sb[:, :])
                    nc.vector.tensor_mul(pa[:, :], pa[:, :], pb[:, :])
                    nxt.append((pa, sa))
                lvl = nxt
            sin = lvl[0][1]

            # ======== scan pass 2: state = pp*seed + sc0; y = state*ces ======
            yT = spool.tile([SC, 512], BF16, tag="yT")
            py = psA.tile([SC, 512], F32, tag="psA")
            for g in range(8):
                stc = work.tile([128, 512], BF16, tag="stc")
                for s in range(2):
                    sl = slice(256 * s, 256 * s + 256)
                    nc.vector.scalar_tensor_tensor(
                        stc[:, sl], ppT[g][:, sl], sin[:, 8 * s + g:8 * s + g + 1],
                        sc0T[g][:, sl], Alu.mult, Alu.add)
                yt = work.tile([128, 512], BF16, tag="yt")
                nc.vector.tensor_mul(yt[:, :], stc[:, :], ces[:, :])
                nc.tensor.matmul(py[:, :], r8S[:, g * 64:(g + 1) * 64], yt[:, :],
                                 start=(g == 0), stop=(g == 7))
            # silu(gate) — single act-table swap, placed after all exp/ln uses
            gate_s = spool.tile([SC, 512], BF16, tag="gate_s")
            nc.scalar.activation(gate_s[:, :], gate_p[:, :], Act.Silu)
            yg = spool.tile([SC, 512], BF16, tag="yg")
            nc.vector.tensor_mul(yg[:, :], py[:, :], gate_s[:, :])

            # ======== out_proj + final residual (hd-major out) ========
            for m in range(8):
                p2 = psB.tile([128, 512], F32, tag="psB")
                nc.tensor.matmul(p2[:, :], outW[:, m * 128:(m + 1) * 128],
                                 yg[:, :], start=True, stop=True)
                x2 = work.tile([128, 512], F32, tag="x2")
                nc.vector.tensor_add(x2[:, :], x1h[m][:, :], p2[:, :])
                nc.sync.dma_start(yout[m * 128:(m + 1) * 128, :], x2[:, :])

    _split_multi_waits(nc)
    return nc


def kernel(x, qkv_w, o_w, norm1_w, norm2_w, in_w, out_w, A_log, Bp_w, Cp_w,
           dt_w, dt_b, gate_w):
    import ml_dtypes
    f = np.float32
    bf = ml_dtypes.bfloat16
    f8 = ml_dtypes.float8_e4m3
    WS = 32.0
    xf = np.ascontiguousarray(np.asarray(x, f).reshape(BT, D))
    xT = np.ascontiguousarray(xf.T)
    xTb = np.ascontiguousarray(xT.astype(bf))
    o_wT = np.ascontiguousarray(np.asarray(o_w, f).T.astype(bf))
    n1 = np.asarray(norm1_w, f)
    n2 = np.asarray(norm2_w, f)
    out_wT = np.ascontiguousarray(np.asarray(out_w, f).T.astype(bf))
    in_wT = np.ascontiguousarray((np.asarray(in_w, f) * n2[None, :]).T.astype(bf))
    gate_wT = np.ascontiguousarray((np.asarray(gate_w, f) * n2[None, :]).T.astype(bf))
    dt_wT = np.ascontiguousarray(np.asarray(dt_w, f).T.astype(bf))
    BpT = np.ascontiguousarray(np.asarray(Bp_w, f).T.astype(bf))
    CpT = np.ascontiguousarray(np.asarray(Cp_w, f).T.astype(bf))
    w1c = np.ascontiguousarray(np.asarray(norm1_w, f).reshape(8, 128).T)
    w2cv = np.ascontiguousarray(np.asarray(norm2_w, f).reshape(8, 128).T)
    dtbv = np.ascontiguousarray(np.asarray(dt_b, f).reshape(SC, 1))
    alogv = np.ascontiguousarray(np.asarray(A_log, f).reshape(1024).reshape(8, 128).T)
    ident = np.eye(128, dtype=f).astype(bf)
    tri_m = (np.arange(128)[None, :] >= np.arange(128)[:, None]).astype(f).astype(bf)
    onesd = np.ones((128, 512), f).astype(bf)
    jj = np.arange(1024)
    escm = (np.arange(SC)[:, None] == (jj[None, :] // 16)).astype(f).astype(bf)
    estm = (np.arange(ST)[:, None] == (np.arange(128)[None, :] % 16)).astype(f).astype(bf)
    r8m = np.zeros((128, 512), f)
    for g in range(8):
        for j in range(128):
            r8m[j, g * 64 + 8 * g + j // 16] = 1.0
    r8m = r8m.astype(bf)
    dselm = np.zeros((16, 1024), f)
    for k in range(8):
        for j in range(128):
            dselm[2 * k + j // 64, k * 128 + j] = 1.0
    dselm = dselm.astype(bf)

    nc = _build()
    in_maps = []
    for c in range(NC):
        h0 = 2 * c
        rows = np.concatenate([np.arange(h0 * 64, (h0 + 2) * 64),
                               D + np.arange(h0 * 64, (h0 + 2) * 64),
                               2 * D + np.arange(h0 * 64, (h0 + 2) * 64)])
        qkw = (np.asarray(qkv_w, f)[rows, :] * n1[None, :])
        qkwT8 = np.ascontiguousarray((qkw[0:256, :].T * WS).astype(f8))
        vwTc = np.ascontiguousarray(qkw[256:384, :].T.astype(bf))
        # my tokens: batch0 [256c, 256c+256) ++ batch1 [256c, 256c+256)
        x_myT = np.ascontiguousarray(np.concatenate(
            [xT[:, 256 * c:256 * c + 256],
             xT[:, T + 256 * c:T + 256 * c + 256]], axis=1))
        sel = (np.arange(NC) < c).astype(f)
        cselv = np.ascontiguousarray(np.tile(sel[None, :], (128, 1)))
        omcv = np.ascontiguousarray(
            np.repeat(1.0 - sel, 16)[None, :].repeat(128, axis=0).astype(f))
        in_maps.append({
            "xTb": xTb, "x_myT": x_myT, "qkwT8": qkwT8, "vwT": vwTc, "o_wT": o_wT,
            "out_wT": out_wT, "in_wT": in_wT, "gate_wT": gate_wT,
            "dt_wT": dt_wT, "BpT": BpT, "CpT": CpT, "w1c": w1c, "w2c": w2cv,
            "dtb": dtbv, "alog": alogv, "ident": ident, "tri": tri_m,
            "onesd": onesd, "esc": escm, "est": estm, "r8": r8m,
            "csel": cselv, "omc": omcv, "dsel": dselm,
            "epsb": np.full((128, 1), EPS, f),
        })
    import os
    trace = bool(int(os.environ.get("BASS_PROFILE", "0")))
    res = run_bass_kernel_spmd(nc, in_maps, core_ids=list(range(NC)),
                               trace=trace)
    if trace:
        print("HW exec time:", res.exec_time_ns, "ns")
        print("trace:", res.instructions_and_trace[1] if res.instructions_and_trace else None)
    out = np.zeros((B, T, D), f)
    for c in range(NC):
        yc = np.asarray(res.results[c]["yout"], f)  # [D, 512]
        out[0, 256 * c:256 * c + 256, :] = yc[:, 0:256].T
        out[1, 256 * c:256 * c + 256, :] = yc[:, 256:512].T
    return out
